# revision 18
# baseline (speedup 1.0000x reference)
"""Trainium2 Bass kernel for nn_ConvInteract (gnn_message_passing).

Math shortcuts (exact, from the reference's structure):
  * The shared edge_index spans only the first S=128 of N=3*B*S=12288
    interleaved node rows.  Nodes >= S have no edges: their GCN output is
    relu(xw + b) (deg=1, self-loop only) and their GRU message is 0.
  * For graph nodes the GatedGraphConv message is the SAME vector for every
    node: m = (sum_i x_i) @ ggc_W[l], so gi = msum @ (ggc_W[l] @ wih.T) with
    the matrix product precomputed on host.
  * sim / norm matrices are symmetric, so column scalings are done as
    (scale rows) -> PE transpose -> (scale rows).

Precision plan (validated vs the fp32 reference in numpy):
  * GRU gate matmuls (the FLOP bulk) run in fp8 e4m3 with DoubleRow perf
    mode (2 contraction rows per instruction = 2x PE throughput per
    contraction).  Scales: activations x16, gru_whh.T x512; the ACT
    evacuation descales by 1/8192.
  * The graph message path (msum @ wgi) and phases A / D stay bf16 -- fp8
    there costs real accuracy.  msbc carries the x8192 factor so the bf16
    message matmuls accumulate into the same (scaled) PSUM banks.

The program is specialized at build time on host-observed input structure
(zero biases / unit gains collapse several vector ops); the build cache is
keyed on those flags and a general fallback path exists for each.

Sharding: node rows are strided-block sharded so every core gets 4 output
row-blocks per output tensor (t), and the graph block (block 0) lands on
core 0.  One SPMD program runs on all 8 cores; a per-core {0,1} mask makes
the graph-path contributions vanish on cores 1..7.
"""

import sys

sys.path.insert(0, '/opt/trn_rl_repo')
sys.path.insert(0, '/root/.axon_site/_ro/trn_rl_repo')

import numpy as np
import ml_dtypes

import concourse.bass as bass
import concourse.tile as tile
from concourse import mybir
from concourse.masks import make_identity

F32 = mybir.dt.float32
BF16 = mybir.dt.bfloat16
FP8 = mybir.dt.float8e4
I32 = mybir.dt.int32
AF = mybir.ActivationFunctionType
ALU = mybir.AluOpType
PM = mybir.MatmulPerfMode
BF = ml_dtypes.bfloat16
F8 = ml_dtypes.float8_e4m3

B, S, D = 32, 128, 768
L = 2
NC = 8
R = 1536            # node rows per core
KI = 6              # contraction chunks (768 / 128)
LN_EPS = 1e-5

XS = 16.0           # fp8 scale for activations
WS = 512.0          # fp8 scale for gru_whh
GS = XS * WS        # combined psum scale in phase C

_TRACE = False
_LAST = None

# ---------------------------------------------------------------- legalizer
_nsplit = [0]


def _legalize_waits(nc, max_waits=1):
    """This container's walrus rejects >1 sem-wait per instruction; hoist
    the excess onto same-engine NOPs placed just before the instruction."""
    for fn in nc.m.functions:
        for bb in fn.blocks:
            out = []
            for inst in bb.instructions:
                si = inst.sync_info
                if si is not None and len(si.on_wait) > max_waits:
                    waits = list(si.on_wait)
                    head, rest = waits[:-max_waits], waits[-max_waits:]
                    for i in range(0, len(head), max_waits):
                        _nsplit[0] += 1
                        out.append(mybir.InstNoOp(
                            name=f"I-waitsplit-{_nsplit[0]}",
                            engine=inst.engine, ins=[], outs=[],
                            sync_info=mybir.SyncInfo(
                                on_wait=head[i:i + max_waits], on_update=[]),
                        ))
                    inst.sync_info = mybir.SyncInfo(
                        on_wait=rest, on_update=list(si.on_update))
                out.append(inst)
            bb.instructions[:] = out


# ------------------------------------------------------------ device program

def _rsqrt_nr(nc, pool, out, vin, iters=2, tag=""):
    """out = 1/sqrt(vin) via bit-trick seed + Newton iterations (no ACT)."""
    p, k = vin.shape
    tmp = pool.tile([p, k], F32, tag="nr_tmp" + tag)
    magic = pool.tile([p, k], I32, tag="nr_magic" + tag)
    nc.vector.memset(magic[:], 0x5f3759df)
    nc.vector.tensor_scalar(out.bitcast(I32)[:], vin.bitcast(I32)[:], 1, None,
                            ALU.logical_shift_right)
    nc.vector.tensor_tensor(out.bitcast(I32)[:], magic[:], out.bitcast(I32)[:],
                            ALU.subtract)
    for _ in range(iters):
        nc.vector.tensor_mul(tmp[:], out[:], out[:])
        nc.vector.tensor_mul(tmp[:], tmp[:], vin[:])
        nc.vector.tensor_scalar(tmp[:], tmp[:], -0.5, 1.5, ALU.mult, ALU.add)
        nc.vector.tensor_mul(out[:], out[:], tmp[:])


def _build_nc(flags):
    bhhn_zero, brz_zero, ln_triv = flags
    nc = bass.Bass("TRN2", target_bir_lowering=False, debug=False,
                   num_devices=NC)

    dt = nc.dram_tensor
    feats = dt("feats", [128, KI, R], FP8, kind="ExternalInput")
    fg_fm = dt("fg_fm", [128, KI, 128], BF16, kind="ExternalInput")
    fg_rm = dt("fg_rm", [128, D], F32, kind="ExternalInput")
    hfeat = dt("hfeat", [128, KI, 512], BF16, kind="ExternalInput")
    gcnw = dt("gcnw", [128, KI, KI, 128], FP8, kind="ExternalInput")
    whht8 = dt("whht8", [128, KI, 3 * D], FP8, kind="ExternalInput")
    wgi = dt("wgi", [128, L, KI, 3 * D], BF16, kind="ExternalInput")
    w1 = dt("w1", [128, 3, KI, D], BF16, kind="ExternalInput")
    w2 = dt("w2", [128, 3, KI, D + 1], BF16, kind="ExternalInput")
    gcnb16 = dt("gcnb16", [128, KI], F32, kind="ExternalInput")
    brz = dt("brz", [128, 2 * KI], F32, kind="ExternalInput")
    bhhn = dt("bhhn", [128, KI], F32, kind="ExternalInput")  # x8192
    bihn = dt("bihn", [128, KI], F32, kind="ExternalInput")
    b1 = dt("b1", [128, 3, KI], F32, kind="ExternalInput")
    b2bf = dt("b2bf", [3, D], BF16, kind="ExternalInput")
    gbf = dt("gbf", [3, D], BF16, kind="ExternalInput")
    bebf = dt("bebf", [3, D], BF16, kind="ExternalInput")
    mb2 = dt("mb2", [128, 3], F32, kind="ExternalInput")
    mask = dt("mask", [128, 1], F32, kind="ExternalInput")
    out = dt("out", [R, D], F32, kind="ExternalOutput")

    def bcast_row(src_2d, t):
        a = src_2d.ap()[t]
        return bass.AP(tensor=a.tensor, offset=a.offset,
                       ap=[[0, 128]] + [list(x) for x in a.ap])

    with tile.TileContext(nc) as tc:
        with tc.tile_pool(name="wp", bufs=1) as wp, \
             tc.tile_pool(name="sh", bufs=3) as sh, \
             tc.tile_pool(name="mw", bufs=2) as mw, \
             tc.tile_pool(name="lw", bufs=1) as lw, \
             tc.tile_pool(name="gp", bufs=2) as gp, \
             tc.tile_pool(name="sp", bufs=1) as sp, \
             tc.tile_pool(name="yp", bufs=2) as yp, \
             tc.tile_pool(name="op", bufs=2) as op, \
             tc.tile_pool(name="ps", bufs=2, space="PSUM") as ps:

            # ---- resident weights / constants (priority DMA order)
            fgf_s = wp.tile([128, KI, 128], BF16, tag="fgf")
            fgr_s = wp.tile([128, D], F32, tag="fgr")
            gcnw_s = wp.tile([128, KI, KI, 128], FP8, tag="gcnw")
            gcnb16_s = wp.tile([128, KI], F32, tag="gcnb16")
            mask_s = wp.tile([128, 1], F32, tag="mask")
            whht_s = wp.tile([128, KI, 3 * D], FP8, tag="whht")
            hf_s = wp.tile([128, KI, 512], BF16, tag="hf")
            brz_s = wp.tile([128, 2 * KI], F32, tag="brz")
            bhhn_s = wp.tile([128, KI], F32, tag="bhhn")
            bihn_s = wp.tile([128, KI], F32, tag="bihn")
            b1_s = wp.tile([128, 3, KI], F32, tag="b1")
            mb2_s = wp.tile([128, 3], F32, tag="mb2")
            # Two HWDGE queues (SP + ACT): phase-A criticals chunked on the
            # SP queue; phase B/C/D weights ride the ACT queue in parallel.
            # All doorbells issue up front -- the queues serialize transfers.
            nc.sync.dma_start(gcnw_s[:, 0], gcnw[:, 0])
            nc.scalar.dma_start(gcnw_s[:, 1:KI], gcnw[:, 1:KI])
            for dst, src in ((fgf_s, fg_fm), (fgr_s, fg_rm),
                             (gcnb16_s, gcnb16), (mask_s, mask),
                             (whht_s, whht8), (brz_s, brz),
                             (bhhn_s, bhhn), (bihn_s, bihn), (b1_s, b1),
                             (mb2_s, mb2)):
                nc.scalar.dma_start(dst[:], src[:])

            def late_hf():
                nc.scalar.dma_start(hf_s[:], hfeat[:])

            def late_dmas():
                pass

            ident = wp.tile([128, 128], F32, tag="ident")
            ones_bf = wp.tile([128, 128], BF16, tag="ones_bf")
            make_identity(nc, ident)
            nc.vector.memset(ones_bf[:], 1.0)

            # state buffers
            x_bf = wp.tile([128, KI, R], BF16, tag="x_bf")
            xf8a = wp.tile([128, KI, R], FP8, tag="xf8a")
            xf8b = wp.tile([128, KI, R], FP8, tag="xf8b")
            xwg_f = wp.tile([128, KI, 128], F32, tag="xwg")
            msbc = wp.tile([128, KI, 128], BF16, tag="msbc")

            # ============ phases A (GCN matmul) + B (graph prelude),
            # ============ interleaved so the tensor queue never stalls.
            ft = [None] * 3

            def phaseA_rt(rt):
                ft[rt] = sh.tile([128, KI, 512], FP8, tag="stream", name=f"ft{rt}")
                eng = nc.scalar if rt == 2 else nc.sync
                eng.dma_start(ft[rt][:], feats[:, :, rt * 512:(rt + 1) * 512])

                for f in range(KI):
                    pxw = ps.tile([128, 512], F32, tag="A")
                    for kp in range(3):
                        k0, k1 = 2 * kp, 2 * kp + 2
                        nc.tensor.matmul(pxw[:], gcnw_s[:, f, k0:k1, :],
                                         ft[rt][:, k0:k1, :], start=(kp == 0),
                                         stop=(kp == 2), perf_mode=PM.DoubleRow,
                                         skip_group_check=True)
                    if rt == 0:
                        nc.vector.tensor_copy(xwg_f[:, f, :], pxw[:, 0:128])
                    # psum carries x16*x512; Relu(psum/512 + 16 b) = 16 relu(xw+b)
                    nc.scalar.activation(xf8a[:, f, rt * 512:(rt + 1) * 512], pxw[:],
                                         AF.Relu, bias=gcnb16_s[:, f:f + 1],
                                         scale=XS / GS)

            # B.0: cosine-norm rsqrt (vector only, needs fgr)
            fgsq = sp.tile([128, D], F32, tag="fgsq")
            nc.vector.tensor_mul(fgsq[:], fgr_s[:], fgr_s[:])
            nrm2 = sp.tile([128, 1], F32, tag="nrm2")
            nc.vector.tensor_reduce(nrm2[:], fgsq[:], mybir.AxisListType.X, ALU.add)
            rn = sp.tile([128, 1], F32, tag="rn")
            _rsqrt_nr(nc, sp, rn, nrm2, iters=3, tag="rn")
            nc.vector.tensor_scalar(rn[:], rn[:], 1e8, None, ALU.min)

            # B.1: gram matmul (needs only fgf -- first DMA to land)
            pg = ps.tile([128, 128], F32, tag="Z")
            for ki in range(KI):
                nc.tensor.matmul(pg[:], fgf_s[:, ki, :], fgf_s[:, ki, :],
                                 start=(ki == 0), stop=(ki == 5))
            gsb = sp.tile([128, 128], F32, tag="gsb")
            nc.vector.tensor_scalar_mul(gsb[:], pg[:], rn[:])        # rn[i]*G

            phaseA_rt(0)

            # B.2: transpose + col scaling -> sim
            ptr = ps.tile([128, 128], F32, tag="Z")
            nc.tensor.transpose(ptr[:], gsb[:], ident[:])
            sim = sp.tile([128, 128], F32, tag="sim")
            nc.vector.tensor_scalar_mul(sim[:], ptr[:], rn[:])       # symmetric

            # B.3: global min / max (reduce -> PE transpose -> reduce -> bcast)
            onesc = sp.tile([1, 128], F32, tag="onesc")
            nc.vector.memset(onesc[:], 1.0)
            smax = sp.tile([128, 1], F32, tag="smax")
            smin = sp.tile([128, 1], F32, tag="smin")
            for which, outt, alu in (("mx", smax, ALU.max), ("mn", smin, ALU.min)):
                rv = sp.tile([128, 1], F32, tag="rv_" + which)
                nc.vector.tensor_reduce(rv[:], sim[:], mybir.AxisListType.X, alu)
                prt = ps.tile([1, 128], F32, tag="Z")
                nc.tensor.transpose(prt[:], rv[:], ident[:])
                rvr = sp.tile([1, 128], F32, tag="rvr_" + which)
                nc.vector.tensor_copy(rvr[:], prt[:])
                gs = sp.tile([1, 1], F32, tag="gs_" + which)
                nc.vector.tensor_reduce(gs[:], rvr[:], mybir.AxisListType.X, alu)
                pbc = ps.tile([128, 1], F32, tag="Z")
                nc.tensor.matmul(pbc[:], onesc[:], gs[:], start=True, stop=True)
                nc.vector.tensor_copy(outt[:], pbc[:])

            phaseA_rt(1)
            late_dmas()

            # B.4: min-max normalize, degrees, norm matrix
            rng = sp.tile([128, 1], F32, tag="rngv")
            nc.vector.tensor_sub(rng[:], smax[:], smin[:])
            rngr = sp.tile([128, 1], F32, tag="rngr")
            nc.vector.reciprocal(rngr[:], rng[:])
            simn = sp.tile([128, 128], F32, tag="simn")
            nc.vector.tensor_scalar(simn[:], sim[:], smin[:], rngr[:],
                                    ALU.subtract, ALU.mult)
            deg = sp.tile([128, 1], F32, tag="deg")
            nc.vector.tensor_reduce(deg[:], simn[:], mybir.AxisListType.X, ALU.add)
            nc.vector.tensor_scalar(deg[:], deg[:], 1.0, None, ALU.add)
            dinv = sp.tile([128, 1], F32, tag="dinv")
            _rsqrt_nr(nc, sp, dinv, deg, iters=3, tag="dinv")
            deginv = sp.tile([128, 1], F32, tag="deginv")
            nc.vector.tensor_mul(deginv[:], dinv[:], dinv[:])
            t1 = sp.tile([128, 128], F32, tag="t1")
            nc.vector.tensor_scalar_mul(t1[:], simn[:], dinv[:])
            pt1 = ps.tile([128, 128], F32, tag="Z")
            nc.tensor.transpose(pt1[:], t1[:], ident[:])
            normM = sp.tile([128, 128], F32, tag="normM")
            nc.vector.tensor_scalar_mul(normM[:], pt1[:], dinv[:])
            diagM = sp.tile([128, 128], F32, tag="diagM")
            nc.vector.tensor_scalar_mul(diagM[:], ident[:], deginv[:])
            nc.vector.tensor_add(normM[:], normM[:], diagM[:])

            phaseA_rt(2)
            late_hf()

            # B.5: xwg -> row-major via PE transpose, agg matmul, exact blend
            xwgr = sp.tile([128, KI, 128], F32, tag="xwgr")
            for f in range(KI):
                ptf = ps.tile([128, 128], F32, tag="Z")
                nc.tensor.transpose(ptf[:], xwg_f[:, f, :], ident[:])
                nc.vector.tensor_copy(xwgr[:, f, :], ptf[:])
            invmask_s = sp.tile([128, 1], F32, tag="invmask")
            nc.vector.tensor_scalar(invmask_s[:], mask_s[:], -1.0, 1.0,
                                    ALU.mult, ALU.add)
            for f in range(KI):
                pa = ps.tile([128, 128], F32, tag="Z")
                nc.tensor.matmul(pa[:], xwgr[:, f, :], normM[:], start=True,
                                 stop=True)
                x0g = sp.tile([128, 128], FP8, tag="x0g")
                nc.scalar.activation(x0g[:], pa[:], AF.Relu,
                                     bias=gcnb16_s[:, f:f + 1], scale=XS / GS)
                dd = sp.tile([128, 128], FP8, tag="dd")
                nc.vector.tensor_scalar_mul(dd[:], x0g[:], mask_s[:])
                nc.vector.tensor_scalar_mul(xf8a[:, f, 0:128],
                                            xf8a[:, f, 0:128], invmask_s[:])
                nc.vector.tensor_add(xf8a[:, f, 0:128], xf8a[:, f, 0:128], dd[:])

            # ---- phase C: 2 GatedGraphConv layers (GRU updates, fp8 gates)
            # rt order (0,1,2) both layers: layer-1 graph cols are ready a
            # third of the way in, so phase D emission interleaves below.
            wgi_s = [None] * L

            def emit_msum(lay):
                x8in = xf8a if lay == 0 else xf8b
                wgi_s[lay] = lw.tile([128, KI, 3 * D], BF16, tag="wgi",
                                     name=f"wgi{lay}")
                if lay == 0:
                    nc.sync.dma_start(wgi_s[lay][:, 0:3], wgi[:, lay, 0:3])
                    nc.scalar.dma_start(wgi_s[lay][:, 3:KI], wgi[:, lay, 3:KI])
                else:
                    nc.sync.dma_start(wgi_s[lay][:], wgi[:, lay])
                msum = sp.tile([128, KI], F32, tag=f"msum{lay}",
                               name=f"msum{lay}")
                for ki in range(KI):
                    nc.vector.tensor_reduce(msum[:, ki:ki + 1],
                                            x8in[:, ki, 0:128],
                                            mybir.AxisListType.X, ALU.add)
                # x8in carries the fp8 x16 scale; fold mask + x8192 in
                nc.vector.tensor_scalar(msum[:], msum[:], mask_s[:], GS / XS,
                                        ALU.mult, ALU.mult)
                for ki in range(KI):
                    nc.vector.tensor_scalar_mul(msbc[:, ki, :], ones_bf[:],
                                                msum[:, ki:ki + 1])

            def emit_C(lay, rt):
                x8in = xf8a if lay == 0 else xf8b
                cs, ce = rt * 512, rt * 512 + 512
                for f in range(KI):
                    # r and z share one [128,1024] psum (2 banks) so a
                    # single ACT can evacuate both when brz == 0.
                    prz = ps.tile([128, 1024], F32, tag="R",
                                  name=f"prz{lay}_{rt}_{f}")
                    pr = prz[:, 0:512]
                    pz = prz[:, 512:1024]
                    pn = ps.tile([128, 512], F32, tag="A",
                                 name=f"pn{lay}_{rt}_{f}")
                    for kp in range(3):
                        k0, k1 = 2 * kp, 2 * kp + 2
                        last = kp == 2
                        nc.tensor.matmul(
                            pr, whht_s[:, k0:k1, f * 128:f * 128 + 128],
                            x8in[:, k0:k1, cs:ce], start=(kp == 0),
                            stop=(last and rt != 0), perf_mode=PM.DoubleRow,
                            skip_group_check=True)
                        nc.tensor.matmul(
                            pz, whht_s[:, k0:k1, D + f * 128:D + f * 128 + 128],
                            x8in[:, k0:k1, cs:ce], start=(kp == 0),
                            stop=(last and rt != 0), perf_mode=PM.DoubleRow,
                            skip_group_check=True)
                        nc.tensor.matmul(
                            pn[:], whht_s[:, k0:k1, 2 * D + f * 128:2 * D + f * 128 + 128],
                            x8in[:, k0:k1, cs:ce], start=(kp == 0),
                            stop=last, perf_mode=PM.DoubleRow,
                            skip_group_check=True)
                    if rt == 0:
                        pgin = ps.tile([128, 128], F32, tag="Z",
                                       name=f"pgin{lay}_{f}")
                        for ki in range(KI):
                            last = ki == 5
                            nc.tensor.matmul(prz[:, 0:128],
                                             wgi_s[lay][:, ki, f * 128:f * 128 + 128],
                                             msbc[:, ki, :], start=False, stop=last,
                                             skip_group_check=True)
                            nc.tensor.matmul(prz[:, 512:640],
                                             wgi_s[lay][:, ki, D + f * 128:D + f * 128 + 128],
                                             msbc[:, ki, :], start=False, stop=last,
                                             skip_group_check=True)
                            nc.tensor.matmul(pgin[:],
                                             wgi_s[lay][:, ki, 2 * D + f * 128:2 * D + f * 128 + 128],
                                             msbc[:, ki, :], start=(ki == 0),
                                             stop=last, skip_group_check=True)
                    rz_sb = gp.tile([128, 1024], BF16, tag="rz",
                                    name=f"rz{lay}_{rt}_{f}")
                    r_sb = rz_sb[:, 0:512]
                    z_sb = rz_sb[:, 512:1024]
                    t_sb = gp.tile([128, 512], BF16, tag="t",
                                   name=f"t{lay}_{rt}_{f}")
                    n_sb = gp.tile([128, 512], BF16, tag="n",
                                   name=f"n{lay}_{rt}_{f}")
                    d_sb = gp.tile([128, 512], BF16, tag="d",
                                   name=f"d{lay}_{rt}_{f}")
                    zd_sb = gp.tile([128, 512], BF16, tag="zd",
                                    name=f"zd{lay}_{rt}_{f}")
                    if brz_zero:
                        nc.scalar.activation(rz_sb[:], prz[:], AF.Sigmoid,
                                             bias=0.0, scale=1.0 / GS)
                    else:
                        nc.scalar.activation(r_sb, pr, AF.Sigmoid,
                                             bias=brz_s[:, f:f + 1],
                                             scale=1.0 / GS)
                        nc.scalar.activation(z_sb, pz, AF.Sigmoid,
                                             bias=brz_s[:, KI + f:KI + f + 1],
                                             scale=1.0 / GS)
                    if bhhn_zero:
                        nc.vector.tensor_mul(t_sb[:], pn[:], r_sb)
                    else:
                        nc.vector.scalar_tensor_tensor(t_sb[:], pn[:],
                                                       bhhn_s[:, f:f + 1],
                                                       r_sb, ALU.add, ALU.mult)
                    if rt == 0:
                        nc.vector.tensor_add(t_sb[:, 0:128], t_sb[:, 0:128],
                                             pgin[:])
                    nc.scalar.activation(n_sb[:], t_sb[:], AF.Tanh,
                                         bias=bihn_s[:, f:f + 1], scale=1.0 / GS)
                    if lay == 0:
                        nc.vector.scalar_tensor_tensor(
                            d_sb[:], x8in[:, f, cs:ce], 1.0 / XS, n_sb[:],
                            ALU.mult, ALU.subtract)
                    else:
                        nc.vector.tensor_sub(d_sb[:], x_bf[:, f, cs:ce],
                                             n_sb[:])
                    nc.vector.tensor_mul(zd_sb[:], z_sb, d_sb[:])
                    nc.vector.tensor_add(x_bf[:, f, cs:ce], n_sb[:], zd_sb[:])
                    if lay == 0:
                        nc.vector.tensor_scalar_mul(xf8b[:, f, cs:ce],
                                                    x_bf[:, f, cs:ce], XS)

            # ---- phase D pieces (emission interleaved with layer 1)
            resid = [None] * 3
            hs = [None] * 3
            mwt = [None] * 3

            def build_resid(rt):
                cs = rt * 512
                resid[rt] = sh.tile([128, KI, 512], BF16, tag="stream",
                                    name=f"resid{rt}")
                for ki in range(KI):
                    nc.vector.scalar_tensor_tensor(resid[rt][:, ki, :],
                                                   x_bf[:, ki, cs:cs + 512],
                                                   0.0, hf_s[:, ki, :],
                                                   ALU.max, ALU.add)

            def emit_W1(rt):
                w1t = mw.tile([128, KI, D], BF16, tag="w1t", name=f"w1t{rt}")
                w2t = mw.tile([128, KI, D + 1], BF16, tag="w2t", name=f"w2t{rt}")
                nc.sync.dma_start(w1t[:], w1[:, rt])
                nc.sync.dma_start(w2t[:], w2[:, rt])
                extra = {}
                if not ln_triv:
                    extra['b2b'] = mw.tile([128, D], BF16, tag="b2b",
                                           name=f"b2b{rt}")
                    extra['gr'] = mw.tile([128, D], BF16, tag="gr",
                                          name=f"gr{rt}")
                    extra['ber'] = mw.tile([128, D], BF16, tag="ber",
                                           name=f"ber{rt}")
                    nc.sync.dma_start(extra['b2b'][:], bcast_row(b2bf, rt))
                    nc.sync.dma_start(extra['gr'][:], bcast_row(gbf, rt))
                    nc.sync.dma_start(extra['ber'][:], bcast_row(bebf, rt))
                mwt[rt] = (w2t, extra)
                h_sb = sh.tile([128, KI, 512], BF16, tag="stream",
                               name=f"h{rt}")
                hs[rt] = h_sb
                for f in range(KI):
                    ph = ps.tile([128, 512], F32, tag="A", name=f"ph{rt}_{f}")
                    for ki in range(KI):
                        nc.tensor.matmul(ph[:], w1t[:, ki, f * 128:f * 128 + 128],
                                         resid[rt][:, ki, :], start=(ki == 0),
                                         stop=(ki == 5))
                    nc.scalar.activation(h_sb[:, f, :], ph[:], AF.Relu,
                                         bias=b1_s[:, rt, f:f + 1], scale=1.0)

            def emit_PY(rt):
                w2t, extra = mwt[rt]
                h_sb = hs[rt]
                ysbs = []
                sQ = yp.tile([128, 4], F32, tag="sQ", name=f"sQ{rt}")
                mu = yp.tile([128, 4], F32, tag="mu", name=f"mu{rt}")
                for rc in range(4):
                    py1 = ps.tile([128, 512], F32, tag="R", name=f"py1_{rt}_{rc}")
                    py2 = ps.tile([128, 257], F32, tag="Z", name=f"py2_{rt}_{rc}")
                    for ki in range(KI):
                        last = ki == 5
                        nc.tensor.matmul(py1[:], h_sb[:, ki, rc * 128:rc * 128 + 128],
                                         w2t[:, ki, 0:512], start=(ki == 0), stop=last,
                                         skip_group_check=True)
                        # col 256 of py2 accumulates h @ rowsum(W2)/D == mean(y)
                        nc.tensor.matmul(py2[:], h_sb[:, ki, rc * 128:rc * 128 + 128],
                                         w2t[:, ki, 512:769], start=(ki == 0), stop=last,
                                         skip_group_check=True)
                    ysb = yp.tile([128, D], BF16, tag="y", bufs=4,
                                  name=f"y{rt}_{rc}")
                    ysbs.append(ysb)
                    nc.vector.tensor_copy(ysb[:, 0:512], py1[:])
                    nc.vector.tensor_copy(ysb[:, 512:768], py2[:, 0:256])
                    nc.vector.tensor_copy(mu[:, rc:rc + 1], py2[:, 256:257])
                    if not ln_triv:
                        nc.vector.tensor_add(ysb[:], ysb[:], extra['b2b'][:])
                    sq = yp.tile([128, D], BF16, tag="sq", name=f"sq{rt}_{rc}")
                    nc.scalar.activation(sq[:], ysb[:], AF.Square,
                                         accum_out=sQ[:, rc:rc + 1])
                # batched LN stats for the 4 row-chunks: [128, 4] ops.
                # For the last rt, stats run per row-pair so the tail is a
                # single short chain instead of one 4-wide barrier.
                if not ln_triv:
                    nc.vector.tensor_scalar(mu[:], mu[:], mb2_s[:, rt:rt + 1],
                                            None, ALU.add)
                mu2 = yp.tile([128, 4], F32, tag="mu2", name=f"mu2{rt}")
                var = yp.tile([128, 4], F32, tag="var", name=f"var{rt}")
                rstd = yp.tile([128, 4], F32, tag="rstd", name=f"rstd{rt}")
                nmr = yp.tile([128, 4], F32, tag="nmr", name=f"nmr{rt}")

                def stats(sl):
                    nc.vector.tensor_mul(mu2[:, sl], mu[:, sl], mu[:, sl])
                    nc.vector.scalar_tensor_tensor(var[:, sl], sQ[:, sl], 1.0 / D,
                                                   mu2[:, sl], ALU.mult,
                                                   ALU.subtract)
                    nc.vector.tensor_scalar(var[:, sl], var[:, sl], LN_EPS,
                                            None, ALU.add)
                    _rsqrt_nr(nc, yp, rstd[:, sl], var[:, sl], iters=2,
                              tag=f"ln{rt}")
                    nc.vector.scalar_tensor_tensor(nmr[:, sl], mu[:, sl], -1.0,
                                                   rstd[:, sl], ALU.mult,
                                                   ALU.mult)

                stats(slice(0, 4))
                for rc in range(4):
                    ysb = ysbs[rc]
                    osb = op.tile([128, D], F32, tag="o", name=f"o{rt}_{rc}")
                    if ln_triv and (rt == 1 or (rt == 2 and rc % 2 == 0)):
                        nc.vector.tensor_scalar(osb[:], ysb[:],
                                                mu[:, rc:rc + 1],
                                                rstd[:, rc:rc + 1],
                                                ALU.subtract, ALU.mult)
                    elif ln_triv:
                        nc.scalar.activation(osb[:], ysb[:], AF.Identity,
                                             bias=nmr[:, rc:rc + 1],
                                             scale=rstd[:, rc:rc + 1])
                    else:
                        t3 = yp.tile([128, D], BF16, tag="t3",
                                     name=f"t3_{rt}_{rc}")
                        nc.scalar.activation(t3[:], ysb[:], AF.Identity,
                                             bias=nmr[:, rc:rc + 1],
                                             scale=rstd[:, rc:rc + 1])
                        t4 = yp.tile([128, D], BF16, tag="t4",
                                     name=f"t4_{rt}_{rc}")
                        nc.vector.tensor_mul(t4[:], t3[:], extra['gr'][:])
                        nc.vector.tensor_add(osb[:], t4[:], extra['ber'][:])
                    row = (rt * 4 + rc) * 128
                    nc.sync.dma_start(out[row:row + 128, :], osb[:])

            # ---- interleaved schedule
            emit_msum(0)
            for rt in (0, 1, 2):
                emit_C(0, rt)
            emit_msum(1)
            emit_C(1, 0)
            emit_C(1, 1)
            build_resid(0)
            emit_W1(0)
            emit_C(1, 2)
            build_resid(1)
            emit_PY(0)
            emit_W1(1)
            build_resid(2)
            emit_W1(2)
            emit_PY(1)
            emit_PY(2)

    _legalize_waits(nc)
    return nc


# ------------------------------------------------------------- host packing

def _fm(x2d):
    """[R, 768] row-major -> [128, 6, R] feature-major."""
    return np.ascontiguousarray(x2d.T.reshape(KI, 128, -1).transpose(1, 0, 2))


def _wlay(w):
    """[768, E] -> [128, 6, E] (contraction on partitions)."""
    return np.ascontiguousarray(w.reshape(KI, 128, -1).transpose(1, 0, 2))


def _fv(v):
    return np.ascontiguousarray(v.reshape(KI, 128).T)


_NC_CACHE = {}


def kernel(**inputs):
    global _LAST
    inp = {k: np.asarray(v, dtype=np.float32) for k, v in inputs.items()}

    feats = np.stack([inp['h_con'].ravel(), inp['h_dep'].ravel(),
                      inp['h_seman'].ravel()], axis=-1).reshape(-1, D)

    bih, bhh = inp['gru_bih'], inp['gru_bhh']
    whht = _wlay(inp['gru_whh'].T * WS)
    wgi_l = [inp['ggc_W'][l] @ inp['gru_wih'].T for l in range(L)]
    wgi = np.stack([_wlay(m) for m in wgi_l], 0).transpose(1, 0, 2, 3)
    w1 = np.stack([_wlay(inp[f'r{i}_W1']) for i in (1, 2, 3)], 0).transpose(1, 0, 2, 3)
    w2 = np.stack(
        [_wlay(np.concatenate([inp[f'r{i}_W2'],
                               inp[f'r{i}_W2'].sum(1, keepdims=True) / D], 1))
         for i in (1, 2, 3)], 0).transpose(1, 0, 2, 3)
    b1 = np.stack([_fv(inp[f'r{i}_b1']) for i in (1, 2, 3)], 0).transpose(1, 0, 2)
    b2v = np.stack([inp[f'r{i}_b2'] for i in (1, 2, 3)], 0)
    gvv = np.stack([inp[f'r{i}_g'] for i in (1, 2, 3)], 0)
    bev = np.stack([inp[f'r{i}_beta'] for i in (1, 2, 3)], 0)
    mb2 = np.broadcast_to(b2v.sum(-1) / D, (128, 3)).astype(np.float32)

    brz_host = (bih + bhh)[0:2 * D]
    flags = (bool(np.all(bhh[2 * D:] == 0.0)),
             bool(np.all(brz_host == 0.0)),
             bool(np.all(b2v == 0.0) and np.all(gvv == 1.0)
                  and np.all(bev == 0.0)))

    common = {
        'fg_fm': _fm(feats[0:128]).astype(BF),
        'fg_rm': feats[0:128].astype(np.float32),
        'gcnw': np.ascontiguousarray((_wlay(inp['gcn_W']) * WS).reshape(128, KI, KI, 128).transpose(0, 2, 1, 3)).astype(F8),
        'whht8': whht.astype(F8),
        'wgi': np.ascontiguousarray(wgi).astype(BF),
        'w1': np.ascontiguousarray(w1).astype(BF),
        'w2': np.ascontiguousarray(w2).astype(BF),
        'gcnb16': (_fv(inp['gcn_b']) * XS).astype(np.float32),
        'brz': np.ascontiguousarray(brz_host.reshape(2 * KI, 128).T).astype(np.float32),
        'bhhn': (_fv(bhh[2 * D:]) * GS).astype(np.float32),
        'bihn': _fv(bih[2 * D:]).astype(np.float32),
        'b1': np.ascontiguousarray(b1).astype(np.float32),
        'b2bf': b2v.astype(BF),
        'gbf': gvv.astype(BF),
        'bebf': bev.astype(BF),
        'mb2': np.ascontiguousarray(mb2),
    }

    in_maps = []
    block_lists = []
    for c in range(NC):
        blocks = [32 * t + 8 * j + c for t in range(3) for j in range(4)]
        block_lists.append(blocks)
        rows = np.concatenate([np.arange(m * 128, m * 128 + 128) for m in blocks])
        fsel = feats[rows]
        hfc = inp['h_feature'][[c, c + 8, c + 16, c + 24]].reshape(512, D)
        m = dict(common)
        m['feats'] = (_fm(fsel) * XS).astype(F8)
        m['hfeat'] = _fm(hfc).astype(BF)
        m['mask'] = np.full((128, 1), 1.0 if c == 0 else 0.0, np.float32)
        in_maps.append(m)

    if flags not in _NC_CACHE:
        _NC_CACHE[flags] = _build_nc(flags)

    from concourse.bass_utils import run_bass_kernel_spmd
    res = run_bass_kernel_spmd(_NC_CACHE[flags], in_maps, core_ids=list(range(NC)),
                               trace=_TRACE)
    _LAST = res

    outs = [np.empty((B, S, D), np.float32) for _ in range(3)]
    for c in range(NC):
        oc = res.results[c]['out']
        for k, mblk in enumerate(block_lists[c]):
            t, b = k // 4, c + 8 * (k % 4)
            outs[t][b] = oc[k * 128:(k + 1) * 128]
    return tuple(outs)
